# revision 12
# baseline (speedup 1.0000x reference)
"""Trainium2 Bass kernel for nn_AdaptiveMask: out = x * ring_mask(current_val).

x: [32, 8, 256, 256] f32.  mask: [256, 256] computed from the scalar
current_val (concentric-ring ramp, values in [0, 1]).

Strategy (memory-bound, pure elementwise):
  - Shard x along batch dim: 4 batches per core across 8 cores (data parallel).
  - Host precomputes the [256, 256] mask from current_val.
  - Tile = one [256, 256] image as an SBUF tile [128, 512]: partition p holds
    image rows 2p and 2p+1 (row-major contiguous), so the SBUF mask tile is
    exactly mask.reshape(128, 512) - 256 KiB, fully unique, no replication.
  - Per core: 32 image tiles. Loads on nc.sync (HWDGE ring 0), multiply on
    DVE in-place, stores on nc.scalar (HWDGE ring 1). Tile framework
    pipelines with a multi-buffer pool.
  - Per-core HBM traffic: 8 MiB in + 0.25 MiB mask + 8 MiB out.
"""

import sys

import numpy as np

for _p in ("/opt/trn_rl_repo",):
    if _p not in sys.path:
        sys.path.append(_p)

from concourse import bacc, bass, tile
from concourse.bass import mybir
from concourse.bass_utils import run_bass_kernel_spmd

N_CORES = 8
B, H, N = 32, 8, 256
MAX_SIZE = 256
RAMP_SIZE = 32

IMGS = (B // N_CORES) * H  # 32 images per core
TILE_P = 128
TILE_F = 512  # one [256, 256] image = [128, 512] f32 = 256 KiB
IPT = 8  # images per data tile: [128, 8, 512] = 2 MiB per DMA

_cache = {}


def _build_program(bufs=3, ipt=IPT):
    nc = bacc.Bacc(None, target_bir_lowering=False)
    x_in = nc.dram_tensor(
        "x_in", [IMGS, TILE_P, TILE_F], mybir.dt.float32, kind="ExternalInput"
    )
    m_in = nc.dram_tensor("m_in", [TILE_P, TILE_F], mybir.dt.float32, kind="ExternalInput")
    out = nc.dram_tensor(
        "out", [IMGS, TILE_P, TILE_F], mybir.dt.float32, kind="ExternalOutput"
    )

    with tile.TileContext(nc) as tc:
        with (
            tc.tile_pool(name="maskp", bufs=1) as mp,
            tc.tile_pool(name="data", bufs=bufs) as dp,
        ):
            # 256 KiB unique mask tile, loaded first on the sync ring.
            # Each data tile packs `ipt` images [128, ipt, 512] via a strided
            # DMA (2 KiB chunks: partition p <- rows 2p, 2p+1 of each image),
            # and the multiply broadcasts the mask over the image dim with a
            # stride-0 AP - no mask replication in SBUF or HBM.
            mt = mp.tile([TILE_P, TILE_F], mybir.dt.float32)
            nc.sync.dma_start(mt[:], m_in[:])
            m_b = mt[:].unsqueeze(1).broadcast_to((TILE_P, ipt, TILE_F))
            for t in range(IMGS // ipt):
                d = dp.tile([TILE_P, ipt, TILE_F], mybir.dt.float32)
                sl = slice(t * ipt, (t + 1) * ipt)
                nc.sync.dma_start(d[:], x_in[sl, :, :].rearrange("i p f -> p i f"))
                nc.vector.tensor_mul(d[:], d[:], m_b)
                nc.scalar.dma_start(out[sl, :, :].rearrange("i p f -> p i f"), d[:])
    nc.finalize()
    return nc


def _get_program():
    if "nc" not in _cache:
        _cache["nc"] = _build_program()
    return _cache["nc"]


def _compute_mask(cv: float) -> np.ndarray:
    """Replicates reference's mask math in numpy f32: [N, N]."""
    template = np.linspace(1.0 - MAX_SIZE, 0.0, MAX_SIZE, dtype=np.float32)
    one_d = np.clip(
        (template + np.float32(cv) * MAX_SIZE) / np.float32(RAMP_SIZE) + np.float32(1.0),
        np.float32(0.0),
        np.float32(1.0),
    ).astype(np.float32)
    one_d = one_d[-(N // 2):]  # [128]
    idx = np.arange(N)
    ring = np.minimum(
        np.minimum(idx[:, None], idx[None, :]),
        np.minimum(N - 1 - idx[:, None], N - 1 - idx[None, :]),
    )  # values in [0, 127] for N=256 — always < N//2, no center special case
    return one_d[ring]


def _run(x, current_val, **spmd_kwargs):
    x = np.ascontiguousarray(np.asarray(x), dtype=np.float32)
    cv = float(np.asarray(current_val).reshape(-1)[0])
    assert x.shape == (B, H, N, N), x.shape

    mask = _compute_mask(cv)  # [256, 256]
    m_t = np.ascontiguousarray(mask.reshape(TILE_P, TILE_F))

    per_core = B // N_CORES
    in_maps = [
        {
            "x_in": x[c * per_core : (c + 1) * per_core].reshape(IMGS, TILE_P, TILE_F),
            "m_in": m_t,
        }
        for c in range(N_CORES)
    ]

    nc = _get_program()
    res = run_bass_kernel_spmd(nc, in_maps, list(range(N_CORES)), **spmd_kwargs)
    out = np.concatenate(
        [r["out"].reshape(per_core, H, N, N) for r in res.results], axis=0
    )
    return out, res


def kernel(x, current_val):
    return _run(x, current_val)[0]


if __name__ == "__main__":
    xs = np.random.randn(B, H, N, N).astype(np.float32)
    cv = np.array([0.1], dtype=np.float32)
    o = kernel(x=xs, current_val=cv)
    expected = xs * _compute_mask(0.1)
    print("self-check max abs diff:", np.abs(o - expected).max())


# revision 28
# speedup vs baseline: 1.6418x; 1.6418x over previous
"""Trainium2 Bass kernel for nn_AdaptiveMask: out = x * ring_mask(current_val).

x: [32, 8, 256, 256] f32.  mask: [256, 256] computed from the scalar
current_val (concentric-ring ramp, values in [0, 1]).

Strategy (memory-bound, pure elementwise):
  - Shard x along batch dim: 4 batches per core across 8 cores (data parallel).
  - Sparse dispatch on mask content (the ring ramp saturates for much of
    cv's range): all-ones -> pure DRAM->DRAM copy program (no mask traffic,
    no multiplies); all-zeros -> memset-store program (no x traffic); else
    the general multiply program.
  - General program: host precomputes the [256, 256] mask from current_val
    and lays it out as a [128, TILE_F] "mega mask" matching the SBUF layout
    of a contiguous chunk of x, so the device does a plain tensor_tensor
    multiply. Contiguous [128, TILE_F] tiles streamed in on nc.sync (HWDGE
    ring 0), multiplied in-place on DVE, streamed out on nc.scalar (HWDGE
    ring 1). Tile framework pipelines via a multi-buffer pool.

Layout math: per-core shard [4, 8, 256, 256] viewed as [(8M/TILE_F/512),
128, TILE_F] row-major. Partition p of every tile holds rpp = TILE_F/256
consecutive image rows starting at image row (p * rpp) % 256, identically
for every tile, so one mega-mask M[p, j] = mask[(p*rpp) % 256 + j//256,
j % 256] serves all tiles.
"""

import sys

import numpy as np

for _p in ("/opt/trn_rl_repo",):
    if _p not in sys.path:
        sys.path.append(_p)

from concourse import bacc, tile
from concourse.bass import mybir
from concourse.bass_utils import run_bass_kernel_spmd

N_CORES = 8
B, H, N = 32, 8, 256
MAX_SIZE = 256
RAMP_SIZE = 32

PER_CORE_ELEMS = (B // N_CORES) * H * N * N  # 2M f32 = 8 MiB
TILE_F = 2048  # free elems per partition per tile (1 MiB tiles)
BUFS = 8  # all 8 tiles resident -> no buffer-reuse stalls

_cache = {}


def _build_program(tile_f=TILE_F, bufs=BUFS, mask_on_scalar=True):
    n_tiles = PER_CORE_ELEMS // (128 * tile_f)
    nc = bacc.Bacc(None, target_bir_lowering=False)
    x_in = nc.dram_tensor(
        "x_in", [n_tiles * 128, tile_f], mybir.dt.float32, kind="ExternalInput"
    )
    m_in = nc.dram_tensor("m_in", [128, tile_f], mybir.dt.float32, kind="ExternalInput")
    out = nc.dram_tensor(
        "out", [n_tiles * 128, tile_f], mybir.dt.float32, kind="ExternalOutput"
    )

    with tile.TileContext(nc) as tc:
        with (
            tc.tile_pool(name="maskp", bufs=1) as mp,
            tc.tile_pool(name="data", bufs=bufs) as dp,
        ):
            mt = mp.tile([128, tile_f], mybir.dt.float32)
            # mask load rides the store (scalar) ring so it overlaps with the
            # first data load on the sync ring; stores only start later.
            meng = nc.scalar if mask_on_scalar else nc.sync
            meng.dma_start(mt[:], m_in[:])
            for t in range(n_tiles):
                d = dp.tile([128, tile_f], mybir.dt.float32)
                nc.sync.dma_start(d[:], x_in[t * 128 : (t + 1) * 128, :])
                nc.vector.tensor_mul(d[:], d[:], mt[:])
                nc.scalar.dma_start(out[t * 128 : (t + 1) * 128, :], d[:])
    nc.finalize()
    return nc


def _get_program(tile_f=TILE_F, bufs=BUFS, mask_on_scalar=True):
    key = (tile_f, bufs, mask_on_scalar)
    if key not in _cache:
        _cache[key] = _build_program(tile_f, bufs, mask_on_scalar)
    return _cache[key]


def _build_copy_program(n_dmas=4):
    """out = x, as DRAM->DRAM copies (used when the mask is all ones).

    Split across the two HWDGE rings so both issue engines share the work.
    """
    nc = bacc.Bacc(None, target_bir_lowering=False)
    rows = PER_CORE_ELEMS // 2048
    x_in = nc.dram_tensor("x_in", [rows, 2048], mybir.dt.float32, kind="ExternalInput")
    out = nc.dram_tensor("out", [rows, 2048], mybir.dt.float32, kind="ExternalOutput")
    with tile.TileContext(nc) as tc:  # noqa: F841 — still need scheduling/sems
        step = rows // n_dmas
        for i in range(n_dmas):
            eng = nc.sync if i % 2 == 0 else nc.scalar
            sl = slice(i * step, (i + 1) * step)
            eng.dma_start(out[sl, :], x_in[sl, :])
    nc.finalize()
    return nc


def _build_zero_program():
    """out = 0 via SBUF memset + broadcast stores (mask all zeros)."""
    nc = bacc.Bacc(None, target_bir_lowering=False)
    rows = PER_CORE_ELEMS // 2048
    out = nc.dram_tensor("out", [rows, 2048], mybir.dt.float32, kind="ExternalOutput")
    with tile.TileContext(nc) as tc:
        with tc.tile_pool(name="z", bufs=1) as zp:
            zt = zp.tile([128, 2048], mybir.dt.float32)
            nc.vector.memset(zt[:], 0.0)
            for t in range(rows // 128):
                eng = nc.sync if t % 2 == 0 else nc.scalar
                eng.dma_start(out[t * 128 : (t + 1) * 128, :], zt[:])
    nc.finalize()
    return nc


def _build_copy_program_raw(n_dmas=4):
    """out = x as DRAM->DRAM copies, raw Bass blocks (no Tile barriers)."""
    nc = bacc.Bacc(None, target_bir_lowering=False)
    rows = PER_CORE_ELEMS // 2048
    x_in = nc.dram_tensor("x_in", [rows, 2048], mybir.dt.float32, kind="ExternalInput")
    out = nc.dram_tensor("out", [rows, 2048], mybir.dt.float32, kind="ExternalOutput")
    step = rows // n_dmas
    evens = [i for i in range(n_dmas) if i % 2 == 0]
    odds = [i for i in range(n_dmas) if i % 2 == 1]
    with (
        nc.Block() as block,
        nc.semaphore("s_dma") as s_sync,
        nc.semaphore("a_dma") as s_act,
    ):

        @block.sync
        def _(sync):
            for i in evens:
                sl = slice(i * step, (i + 1) * step)
                sync.dma_start(out[sl, :], x_in[sl, :]).then_inc(s_sync, 16)
            sync.wait_ge(s_sync, 16 * len(evens))

        @block.scalar
        def _(scalar):
            for i in odds:
                sl = slice(i * step, (i + 1) * step)
                scalar.dma_start(out[sl, :], x_in[sl, :]).then_inc(s_act, 16)
            scalar.wait_ge(s_act, 16 * len(odds))

    nc.finalize()
    return nc


def _get_special_program(kind, n_dmas=4, raw=False):
    key = ("special", kind, n_dmas, raw)
    if key not in _cache:
        if kind == "copy":
            build = _build_copy_program_raw if raw else _build_copy_program
            _cache[key] = build(n_dmas)
        else:
            _cache[key] = _build_zero_program()
    return _cache[key]


def _compute_mask(cv: float) -> np.ndarray:
    """Replicates reference's mask math in numpy f32: [N, N]."""
    template = np.linspace(1.0 - MAX_SIZE, 0.0, MAX_SIZE, dtype=np.float32)
    one_d = np.clip(
        (template + np.float32(cv) * MAX_SIZE) / np.float32(RAMP_SIZE) + np.float32(1.0),
        np.float32(0.0),
        np.float32(1.0),
    ).astype(np.float32)
    one_d = one_d[-(N // 2):]  # [128]
    idx = np.arange(N)
    ring = np.minimum(
        np.minimum(idx[:, None], idx[None, :]),
        np.minimum(N - 1 - idx[:, None], N - 1 - idx[None, :]),
    )  # values in [0, 127] for N=256 — always < N//2, no center special case
    return one_d[ring]


def _mega_mask(mask: np.ndarray, tile_f: int) -> np.ndarray:
    """[128, tile_f] mask matching the SBUF layout of a contiguous x tile."""
    rpp = tile_f // N  # image rows per partition
    rows = (np.arange(128)[:, None] * rpp) % N + np.arange(tile_f)[None, :] // N
    cols = np.arange(tile_f)[None, :] % N
    return np.ascontiguousarray(mask[rows, cols])


def _run(x, current_val, tile_f=TILE_F, bufs=BUFS, mask_on_scalar=True,
         allow_special=True, **spmd_kwargs):
    n_dmas = spmd_kwargs.pop("n_dmas", 4)
    raw = spmd_kwargs.pop("raw", False)
    x = np.ascontiguousarray(np.asarray(x), dtype=np.float32)
    cv = float(np.asarray(current_val).reshape(-1)[0])
    assert x.shape == (B, H, N, N), x.shape

    mask = _compute_mask(cv)  # [256, 256]
    per_core = B // N_CORES

    # Sparse dispatch: the ring ramp saturates for much of cv's range -
    # all-ones (x * 1 = x -> pure copy, no mask traffic or multiplies) and
    # all-zeros (-> memset stores, no x traffic at all) have dedicated
    # programs. The general program handles everything else.
    special = None
    if allow_special:
        if mask.min() >= 1.0:
            special = "copy"
        elif mask.max() <= 0.0:
            special = "zero"

    if special == "copy":
        nc = _get_special_program("copy", n_dmas, raw)
        rows = PER_CORE_ELEMS // 2048
        in_maps = [
            {"x_in": x[c * per_core : (c + 1) * per_core].reshape(rows, 2048)}
            for c in range(N_CORES)
        ]
    elif special == "zero":
        nc = _get_special_program("zero")
        in_maps = [{} for _ in range(N_CORES)]
    else:
        nc = _get_program(tile_f, bufs, mask_on_scalar)
        m_t = _mega_mask(mask, tile_f)
        n_rows = PER_CORE_ELEMS // tile_f
        in_maps = [
            {
                "x_in": x[c * per_core : (c + 1) * per_core].reshape(n_rows, tile_f),
                "m_in": m_t,
            }
            for c in range(N_CORES)
        ]

    res = run_bass_kernel_spmd(nc, in_maps, list(range(N_CORES)), **spmd_kwargs)
    out = np.concatenate(
        [r["out"].reshape(per_core, H, N, N) for r in res.results], axis=0
    )
    return out, res


def kernel(x, current_val):
    return _run(x, current_val)[0]


if __name__ == "__main__":
    xs = np.random.randn(B, H, N, N).astype(np.float32)
    cv = np.array([0.1], dtype=np.float32)
    o = kernel(x=xs, current_val=cv)
    expected = xs * _compute_mask(0.1)
    print("self-check max abs diff:", np.abs(o - expected).max())


# revision 32
# speedup vs baseline: 1.8037x; 1.0986x over previous
"""Trainium2 Bass kernel for nn_AdaptiveMask: out = x * ring_mask(current_val).

x: [32, 8, 256, 256] f32.  mask: [256, 256] computed from the scalar
current_val (concentric-ring ramp, values in [0, 1]).

Strategy (memory-bound, pure elementwise):
  - Shard x along batch dim: 4 batches per core across 8 cores (data parallel).
  - Sparse dispatch on mask content (the ring ramp saturates for much of
    cv's range): all-ones -> pure DRAM->DRAM copy program (no mask traffic,
    no multiplies); all-zeros -> memset-store program (no x traffic); else
    the general multiply program.
  - General program: host precomputes the [256, 256] mask from current_val
    and lays it out as a [128, TILE_F] "mega mask" matching the SBUF layout
    of a contiguous chunk of x, so the device does a plain tensor_tensor
    multiply. Contiguous [128, TILE_F] tiles streamed in on nc.sync (HWDGE
    ring 0), multiplied in-place on DVE, streamed out on nc.scalar (HWDGE
    ring 1). Tile framework pipelines via a multi-buffer pool.

Layout math: per-core shard [4, 8, 256, 256] viewed as [(8M/TILE_F/512),
128, TILE_F] row-major. Partition p of every tile holds rpp = TILE_F/256
consecutive image rows starting at image row (p * rpp) % 256, identically
for every tile, so one mega-mask M[p, j] = mask[(p*rpp) % 256 + j//256,
j % 256] serves all tiles.
"""

import sys

import numpy as np

for _p in ("/opt/trn_rl_repo",):
    if _p not in sys.path:
        sys.path.append(_p)

from concourse import bacc, tile
from concourse.bass import mybir
from concourse.bass_utils import run_bass_kernel_spmd

N_CORES = 8
B, H, N = 32, 8, 256
MAX_SIZE = 256
RAMP_SIZE = 32

PER_CORE_ELEMS = (B // N_CORES) * H * N * N  # 2M f32 = 8 MiB
TILE_F = 2048  # free elems per partition per tile (1 MiB tiles)
BUFS = 8  # all 8 tiles resident -> no buffer-reuse stalls
TAIL_SPLIT = 1  # column chunks for the final tile's load/mul/store chain

_cache = {}


def _build_program(tile_f=TILE_F, bufs=BUFS, mask_on_scalar=True, tail_split=1):
    n_tiles = PER_CORE_ELEMS // (128 * tile_f)
    nc = bacc.Bacc(None, target_bir_lowering=False)
    x_in = nc.dram_tensor(
        "x_in", [n_tiles * 128, tile_f], mybir.dt.float32, kind="ExternalInput"
    )
    m_in = nc.dram_tensor("m_in", [128, tile_f], mybir.dt.float32, kind="ExternalInput")
    out = nc.dram_tensor(
        "out", [n_tiles * 128, tile_f], mybir.dt.float32, kind="ExternalOutput"
    )

    with tile.TileContext(nc) as tc:
        with (
            tc.tile_pool(name="maskp", bufs=1) as mp,
            tc.tile_pool(name="data", bufs=bufs) as dp,
        ):
            mt = mp.tile([128, tile_f], mybir.dt.float32)
            # mask load rides the store (scalar) ring so it overlaps with the
            # first data load on the sync ring; stores only start later.
            meng = nc.scalar if mask_on_scalar else nc.sync
            meng.dma_start(mt[:], m_in[:])
            for t in range(n_tiles):
                rs = slice(t * 128, (t + 1) * 128)
                d = dp.tile([128, tile_f], mybir.dt.float32)
                if t == n_tiles - 1 and tail_split > 1:
                    # chunk the final tile so the tail load->mul->store chain
                    # is short (it sits on the critical path after the last
                    # full load completes)
                    w = tile_f // tail_split
                    for s in range(tail_split):
                        cs = slice(s * w, (s + 1) * w)
                        nc.sync.dma_start(d[:, cs], x_in[rs, cs])
                        nc.vector.tensor_mul(d[:, cs], d[:, cs], mt[:, cs])
                        nc.scalar.dma_start(out[rs, cs], d[:, cs])
                else:
                    nc.sync.dma_start(d[:], x_in[rs, :])
                    nc.vector.tensor_mul(d[:], d[:], mt[:])
                    nc.scalar.dma_start(out[rs, :], d[:])
    nc.finalize()
    return nc


def _get_program(tile_f=TILE_F, bufs=BUFS, mask_on_scalar=True, tail_split=1):
    key = (tile_f, bufs, mask_on_scalar, tail_split)
    if key not in _cache:
        _cache[key] = _build_program(tile_f, bufs, mask_on_scalar, tail_split)
    return _cache[key]


def _build_copy_program(n_dmas=4):
    """out = x, as DRAM->DRAM copies (used when the mask is all ones).

    Split across the two HWDGE rings so both issue engines share the work.
    """
    nc = bacc.Bacc(None, target_bir_lowering=False)
    rows = PER_CORE_ELEMS // 2048
    x_in = nc.dram_tensor("x_in", [rows, 2048], mybir.dt.float32, kind="ExternalInput")
    out = nc.dram_tensor("out", [rows, 2048], mybir.dt.float32, kind="ExternalOutput")
    with tile.TileContext(nc) as tc:  # noqa: F841 — still need scheduling/sems
        step = rows // n_dmas
        for i in range(n_dmas):
            eng = nc.sync if i % 2 == 0 else nc.scalar
            sl = slice(i * step, (i + 1) * step)
            eng.dma_start(out[sl, :], x_in[sl, :])
    nc.finalize()
    return nc


def _build_zero_program():
    """out = 0 via SBUF memset + broadcast stores (mask all zeros)."""
    nc = bacc.Bacc(None, target_bir_lowering=False)
    rows = PER_CORE_ELEMS // 2048
    out = nc.dram_tensor("out", [rows, 2048], mybir.dt.float32, kind="ExternalOutput")
    with tile.TileContext(nc) as tc:
        with tc.tile_pool(name="z", bufs=1) as zp:
            zt = zp.tile([128, 2048], mybir.dt.float32)
            nc.vector.memset(zt[:], 0.0)
            for t in range(rows // 128):
                eng = nc.sync if t % 2 == 0 else nc.scalar
                eng.dma_start(out[t * 128 : (t + 1) * 128, :], zt[:])
    nc.finalize()
    return nc


def _build_copy_program_raw(n_dmas=4):
    """out = x as DRAM->DRAM copies, raw Bass blocks (no Tile barriers)."""
    nc = bacc.Bacc(None, target_bir_lowering=False)
    rows = PER_CORE_ELEMS // 2048
    x_in = nc.dram_tensor("x_in", [rows, 2048], mybir.dt.float32, kind="ExternalInput")
    out = nc.dram_tensor("out", [rows, 2048], mybir.dt.float32, kind="ExternalOutput")
    step = rows // n_dmas
    evens = [i for i in range(n_dmas) if i % 2 == 0]
    odds = [i for i in range(n_dmas) if i % 2 == 1]
    with (
        nc.Block() as block,
        nc.semaphore("s_dma") as s_sync,
        nc.semaphore("a_dma") as s_act,
    ):

        @block.sync
        def _(sync):
            for i in evens:
                sl = slice(i * step, (i + 1) * step)
                sync.dma_start(out[sl, :], x_in[sl, :]).then_inc(s_sync, 16)
            sync.wait_ge(s_sync, 16 * len(evens))

        @block.scalar
        def _(scalar):
            for i in odds:
                sl = slice(i * step, (i + 1) * step)
                scalar.dma_start(out[sl, :], x_in[sl, :]).then_inc(s_act, 16)
            scalar.wait_ge(s_act, 16 * len(odds))

    nc.finalize()
    return nc


def _get_special_program(kind, n_dmas=4, raw=False):
    key = ("special", kind, n_dmas, raw)
    if key not in _cache:
        if kind == "copy":
            build = _build_copy_program_raw if raw else _build_copy_program
            _cache[key] = build(n_dmas)
        else:
            _cache[key] = _build_zero_program()
    return _cache[key]


def _compute_mask(cv: float) -> np.ndarray:
    """Replicates reference's mask math in numpy f32: [N, N]."""
    template = np.linspace(1.0 - MAX_SIZE, 0.0, MAX_SIZE, dtype=np.float32)
    one_d = np.clip(
        (template + np.float32(cv) * MAX_SIZE) / np.float32(RAMP_SIZE) + np.float32(1.0),
        np.float32(0.0),
        np.float32(1.0),
    ).astype(np.float32)
    one_d = one_d[-(N // 2):]  # [128]
    idx = np.arange(N)
    ring = np.minimum(
        np.minimum(idx[:, None], idx[None, :]),
        np.minimum(N - 1 - idx[:, None], N - 1 - idx[None, :]),
    )  # values in [0, 127] for N=256 — always < N//2, no center special case
    return one_d[ring]


def _mega_mask(mask: np.ndarray, tile_f: int) -> np.ndarray:
    """[128, tile_f] mask matching the SBUF layout of a contiguous x tile."""
    rpp = tile_f // N  # image rows per partition
    rows = (np.arange(128)[:, None] * rpp) % N + np.arange(tile_f)[None, :] // N
    cols = np.arange(tile_f)[None, :] % N
    return np.ascontiguousarray(mask[rows, cols])


def _run(x, current_val, tile_f=TILE_F, bufs=BUFS, mask_on_scalar=True,
         allow_special=True, **spmd_kwargs):
    n_dmas = spmd_kwargs.pop("n_dmas", 4)
    raw = spmd_kwargs.pop("raw", False)
    tail_split = spmd_kwargs.pop("tail_split", TAIL_SPLIT)
    x = np.ascontiguousarray(np.asarray(x), dtype=np.float32)
    cv = float(np.asarray(current_val).reshape(-1)[0])
    assert x.shape == (B, H, N, N), x.shape

    mask = _compute_mask(cv)  # [256, 256]
    per_core = B // N_CORES

    # Sparse dispatch: the ring ramp saturates for much of cv's range -
    # all-ones (x * 1 = x -> pure copy, no mask traffic or multiplies) and
    # all-zeros (-> memset stores, no x traffic at all) have dedicated
    # programs. The general program handles everything else.
    special = None
    if allow_special:
        if mask.min() >= 1.0:
            special = "copy"
        elif mask.max() <= 0.0:
            special = "zero"

    if special == "copy":
        nc = _get_special_program("copy", n_dmas, raw)
        rows = PER_CORE_ELEMS // 2048
        in_maps = [
            {"x_in": x[c * per_core : (c + 1) * per_core].reshape(rows, 2048)}
            for c in range(N_CORES)
        ]
    elif special == "zero":
        nc = _get_special_program("zero")
        in_maps = [{} for _ in range(N_CORES)]
    else:
        nc = _get_program(tile_f, bufs, mask_on_scalar, tail_split)
        m_t = _mega_mask(mask, tile_f)
        n_rows = PER_CORE_ELEMS // tile_f
        in_maps = [
            {
                "x_in": x[c * per_core : (c + 1) * per_core].reshape(n_rows, tile_f),
                "m_in": m_t,
            }
            for c in range(N_CORES)
        ]

    res = run_bass_kernel_spmd(nc, in_maps, list(range(N_CORES)), **spmd_kwargs)
    out = np.concatenate(
        [r["out"].reshape(per_core, H, N, N) for r in res.results], axis=0
    )
    return out, res


def kernel(x, current_val):
    return _run(x, current_val)[0]


if __name__ == "__main__":
    xs = np.random.randn(B, H, N, N).astype(np.float32)
    cv = np.array([0.1], dtype=np.float32)
    o = kernel(x=xs, current_val=cv)
    expected = xs * _compute_mask(0.1)
    print("self-check max abs diff:", np.abs(o - expected).max())
